# revision 26
# baseline (speedup 1.0000x reference)
"""Trainium2 Bass kernel for nn_Attention_8744553414813.

Reference computation (B=4, C=512, H=W=64, HW=4096):
    Q = conv1x1(mean_norm(content), Wq, bq)   # [B, C, HW]
    K = conv1x1(mean_norm(style),   Wk, bk)
    V = conv1x1(style,              Wv, bv)
    A = softmax(Q^T K, axis=-1)               # [B, HWc, HWs]
    out = V @ A^T                             # [B, C, HW]

Sharding: 8 cores = 4 batches x 2 content-pixel halves (data parallel; the
small 1x1-conv weights are replicated). Each core computes out^T for its
2048 query pixels; the host transposes and reassembles.

Per-core device program:
 - channel mean/var via bn_stats over streamed fp32 chunks
 - normalization folded into the conv weights: W' = W*diag(1/std),
   b' = b - W' @ mean  (so the projections consume RAW inputs)
 - whole matmul pipeline in fp16 (output error is dominated by softmax
   near-ties, which fp16 vs fp32r does not change; fp16 matmuls run at
   full PE rate with overlapped weight loads)
 - projection inputs loaded via gpsimd casting DMA (fp32 DRAM -> fp16 SBUF)
 - flash-style online softmax over four 1024-col PSUM chunks; exp on the
   scalar engine with per-partition bias and fused row-sum (accum_out)
 - A^T produced by SBUF->SBUF DMA transpose on the second HWDGE ring
 - AV matmul (fp16) accumulates out^T [q, c] directly
"""
import numpy as np

import concourse.bacc as bacc
import concourse.bass as bass
import concourse.mybir as mybir
import concourse.tile as tile
from concourse.bass_utils import run_bass_kernel_spmd

F32 = mybir.dt.float32
F16 = mybir.dt.float16
AF = mybir.ActivationFunctionType
AX = mybir.AxisListType
OP = mybir.AluOpType

B, C, H, W = 4, 512, 64, 64
HW = H * W                  # 4096 (style/key pixels per core)
QN = HW // 2                # 2048 query pixels per core
CS = C // 128               # 4 channel sub-tiles
EPS = 1e-5
KCHUNK = 1024               # scores psum chunk width (2 banks)
NKC = HW // KCHUNK          # 4 online-softmax chunks
PIX = 512                   # projection pixel chunk


def dram_chunk(x, t):
    """[C, HW] dram slice -> [128, CS, PIX] chunk t."""
    return x[:, t * PIX:(t + 1) * PIX].rearrange("(co ci) f -> ci co f", ci=128)


def build_nc():
    nc = bacc.Bacc(trn_type="TRN2")
    xc = nc.dram_tensor("xc", [C, HW], F32, kind="ExternalInput")      # content (full batch)
    xs = nc.dram_tensor("xs", [C, HW], F32, kind="ExternalInput")      # style
    wq = nc.dram_tensor("wq_t", [C, C], F32, kind="ExternalInput")     # Wq^T [cin, cout]
    wk = nc.dram_tensor("wk_t", [C, C], F32, kind="ExternalInput")
    wv = nc.dram_tensor("wv_t", [C, C], F32, kind="ExternalInput")
    bq = nc.dram_tensor("bq_p", [128, CS], F32, kind="ExternalInput")  # bias packed [p, sub]
    bk = nc.dram_tensor("bk_p", [128, CS], F32, kind="ExternalInput")
    bv = nc.dram_tensor("bv_v", [C], F32, kind="ExternalInput")
    out = nc.dram_tensor("out_t", [QN, C], F32, kind="ExternalOutput")  # out^T for this core

    with tile.TileContext(nc) as tc:
        with tc.tile_pool(name="sb", bufs=1) as sb, \
             tc.tile_pool(name="cst", bufs=1) as cst, \
             tc.tile_pool(name="chk", bufs=2) as chk, \
             tc.tile_pool(name="wr", bufs=1) as wrp, \
             tc.tile_pool(name="qc", bufs=2) as qcp, \
             tc.tile_pool(name="ab", bufs=2) as abp, \
             tc.tile_pool(name="atb", bufs=1) as atp, \
             tc.tile_pool(name="ob", bufs=3) as obp, \
             tc.tile_pool(name="sm", bufs=2) as smp, \
             tc.tile_pool(name="psS", bufs=2, space="PSUM") as psS, \
             tc.tile_pool(name="psT", bufs=2, space="PSUM") as psT, \
             tc.tile_pool(name="psM", bufs=2, space="PSUM") as psM:

            # ---------- constants ----------
            from concourse.masks import make_identity
            ident = cst.tile([128, 128], F16)
            make_identity(nc, ident)
            eps_t = cst.tile([128, 1], F32)
            nc.vector.memset(eps_t[:], EPS)
            bq_t = cst.tile([128, CS], F32)
            nc.sync.dma_start(bq_t[:], bq[:])
            bk_t = cst.tile([128, CS], F32)
            nc.sync.dma_start(bk_t[:], bk[:])
            bvap = bv[:]
            bv_t = cst.tile([128, C], F32)
            nc.gpsimd.dma_start(
                bv_t[:],
                bass.AP(tensor=bvap.tensor, offset=bvap.offset, ap=[[0, 128]] + list(bvap.ap)),
            )

            # raw V weights: sync-ring fp32 load + ACT cast to fp16
            wvf = chk.tile([128, CS, C], F32, tag="chk32")
            nc.sync.dma_start(wvf[:], wv.rearrange("(co ci) o -> ci co o", ci=128))
            wv_r = wrp.tile([128, CS, C], F16, tag="wvr")
            nc.scalar.copy(wv_r[:], wvf[:])

            # ---------- resident fp16 activations: sync fp32 loads + ACT casts ----------
            xs_h = sb.tile([128, CS, HW], F16)               # style, 32 KB/p
            xc_h = sb.tile([128, CS, HW], F16)               # content, 32 KB/p
            vt = sb.tile([128, HW // 128, C], F16)           # V^T [k, cout], 32 KB/p
            st_s = cst.tile([128, CS, HW // PIX, 6], F32)
            st_c = cst.tile([128, CS, HW // PIX, 6], F32)

            # style stream: cast + stats + V^T projection per chunk
            for t in range(HW // PIX):
                xsf = chk.tile([128, CS, PIX], F32, tag="chk32")
                nc.sync.dma_start(xsf[:], dram_chunk(xs, t))
                nc.scalar.copy(xs_h[:, :, t * PIX:(t + 1) * PIX], xsf[:])
                for sub in range(CS):
                    nc.vector.bn_stats(st_s[:, sub, t, :], xs_h[:, sub, t * PIX:(t + 1) * PIX])
                for ks in range(PIX // 128):
                    koff = t * PIX + ks * 128
                    psv = psM.tile([128, C], F32, tag="mm512")
                    for sub in range(CS):
                        nc.tensor.matmul(psv[:], xs_h[:, sub, koff:koff + 128],
                                         wv_r[:, sub, :], start=(sub == 0), stop=(sub == CS - 1))
                    nc.scalar.copy(vt[:, t * (PIX // 128) + ks, :], psv[:])
            # raw Q/K weights: loaded now (right behind the style stream) and held
            # in fp32 until their folds
            wraw = {}
            for name, t in (("k", wk), ("q", wq)):
                wf = chk.tile([128, CS, C], F32, tag="chk32", name=f"wf_{name}")
                nc.sync.dma_start(wf[:], t.rearrange("(co ci) o -> ci co o", ci=128))
                wraw[name] = wf

            # content stream: cast + stats
            for t in range(HW // PIX):
                xcf = chk.tile([128, CS, PIX], F32, tag="chk32")
                nc.sync.dma_start(xcf[:], dram_chunk(xc, t))
                nc.scalar.copy(xc_h[:, :, t * PIX:(t + 1) * PIX], xcf[:])
                for sub in range(CS):
                    nc.vector.bn_stats(st_c[:, sub, t, :], xc_h[:, sub, t * PIX:(t + 1) * PIX])

            # ---------- finalize stats; fold weights ----------
            folded = {}
            for name, st, wname in (("s", st_s, "k"), ("c", st_c, "q")):
                mv = cst.tile([128, CS, 2], F32, tag=f"mv_{name}")
                for sub in range(CS):
                    nc.vector.bn_aggr(mv[:, sub, :], st[:, sub, :, :])
                mean_h = cst.tile([128, CS], F16, tag=f"meanh_{name}")
                nc.vector.tensor_copy(mean_h[:], mv[:, :, 0])
                std = cst.tile([128, CS], F32, tag=f"std_{name}")
                nc.scalar.activation(std[:], mv[:, :, 1], AF.Sqrt,
                                     bias=eps_t[:], scale=float(HW) / (HW - 1))
                rstd = cst.tile([128, CS], F32, tag=f"rstd_{name}")
                nc.vector.reciprocal(rstd[:], std[:])
                w_r = wrp.tile([128, CS, C], F16, tag=f"w_{wname}")
                for sub in range(CS):
                    nc.vector.tensor_scalar_mul(w_r[:, sub, :], wraw[wname][:, sub, :],
                                                rstd[:, sub:sub + 1])
                folded[wname] = (w_r, mean_h)

            beff = {}
            for wname, bt in (("k", bk_t), ("q", bq_t)):
                # s[cout] = sum_c W'^T[c, cout] * mean[c] as a [1, 512] row,
                # then partition-scattered to [128, CS]
                w_r, mean_h = folded[wname]
                pbrow = psM.tile([128, C], F32, tag="mm512")
                for ci in range(CS):
                    nc.tensor.matmul(pbrow[0:1, :], mean_h[:, ci:ci + 1], w_r[:, ci, :],
                                     start=(ci == 0), stop=(ci == CS - 1))
                srow = cst.tile([1, C], F32, tag=f"srow_{wname}")
                nc.vector.tensor_copy(srow[:], pbrow[0:1, :])
                ssc = cst.tile([128, CS], F32, tag=f"ssc_{wname}")
                for s in range(CS):
                    nc.sync.dma_start(ssc[:, s:s + 1], srow[0:1, s * 128:(s + 1) * 128])
                be = cst.tile([128, CS], F32, tag=f"beff_{wname}")
                nc.vector.tensor_tensor(be[:], bt[:], ssc[:], OP.subtract)
                beff[wname] = be

            # ---------- K projection ----------
            wk_r, _ = folded["k"]
            kt = sb.tile([128, CS, HW], F16)                 # K [cout, k], 32 KB/p
            for t in range(HW // PIX):
                for co in range(CS):
                    psk = psM.tile([128, PIX], F32, tag="mm512")
                    for ci in range(CS):
                        nc.tensor.matmul(psk[:], wk_r[:, ci, co * 128:(co + 1) * 128],
                                         xs_h[:, ci, t * PIX:(t + 1) * PIX],
                                         start=(ci == 0), stop=(ci == CS - 1))
                    nc.scalar.activation(kt[:, co, t * PIX:(t + 1) * PIX], psk[:],
                                         AF.Identity, bias=beff["k"][:, co:co + 1], scale=1.0)

            # ---------- Q projection + attention ----------
            wq_r, _ = folded["q"]
            pend = None   # (at, rd, q0) of the previous q-tile

            def flush(p):
                # transpose + AV + epilogue of a finished q-tile; emitted after
                # the NEXT tile's score matmuls so the PE can fill the softmax
                # tail with this work
                at_p, rd_p, q0_p = p
                att = atp.tile([128, HW // 128, 128], F16, tag="AT")
                for g in range(HW // 128 // 8):
                    tp = psT.tile([128, 1024], F16, tag="tp")
                    for i in range(8):
                        kb = g * 8 + i
                        nc.tensor.transpose(tp[:, i * 128:(i + 1) * 128],
                                            at_p[:, kb * 128:(kb + 1) * 128], ident[:])
                    nc.scalar.copy(att[:, g * 8:(g + 1) * 8, :], tp[:])
                av = psM.tile([128, C], F32, tag="mm512")
                for kb in range(HW // 128):
                    nc.tensor.matmul(av[:], att[:, kb, :], vt[:, kb, :],
                                     start=(kb == 0), stop=(kb == HW // 128 - 1))
                ot = obp.tile([128, C], F32, tag="ot")
                nc.vector.tensor_scalar_mul(ot[:], av[:], rd_p[:])
                nc.vector.tensor_tensor(ot[:], ot[:], bv_t[:], OP.add)
                nc.sync.dma_start(out[q0_p:q0_p + 128, :], ot[:])

            for t in range(QN // PIX):
                qc = qcp.tile([128, CS, PIX], F16, tag="qc")
                for co in range(CS):
                    psq = psM.tile([128, PIX], F32, tag="mm512")
                    for ci in range(CS):
                        nc.tensor.matmul(psq[:], wq_r[:, ci, co * 128:(co + 1) * 128],
                                         xc_h[:, ci, t * PIX:(t + 1) * PIX],
                                         start=(ci == 0), stop=(ci == CS - 1))
                    nc.scalar.activation(qc[:, co, :], psq[:],
                                         AF.Identity, bias=beff["q"][:, co:co + 1], scale=1.0)

                for j in range(PIX // 128):          # q-tile of 128 queries
                    at = abp.tile([128, HW], F16, tag="A")
                    mruns = smp.tile([128, NKC], F32, tag="mruns")
                    negs = smp.tile([128, NKC], F32, tag="negs")
                    dvec = smp.tile([128, NKC], F32, tag="dvec")
                    for kc in range(NKC):
                        sps = psS.tile([128, KCHUNK], F32, tag="s")
                        for kb in range(KCHUNK // PIX):
                            koff = kc * KCHUNK + kb * PIX
                            for sub in range(CS):
                                nc.tensor.matmul(sps[:, kb * PIX:(kb + 1) * PIX],
                                                 qc[:, sub, j * 128:(j + 1) * 128],
                                                 kt[:, sub, koff:koff + PIX],
                                                 start=(sub == 0), stop=(sub == CS - 1))
                        if kc == 0:
                            nc.vector.reduce_max(mruns[:, 0:1], sps[:], axis=AX.X)
                        else:
                            mx = smp.tile([128, 1], F32, tag="mx")
                            nc.vector.reduce_max(mx[:], sps[:], axis=AX.X)
                            nc.vector.tensor_tensor(mruns[:, kc:kc + 1], mruns[:, kc - 1:kc],
                                                    mx[:], OP.max)
                        nc.vector.tensor_scalar_mul(negs[:, kc:kc + 1], mruns[:, kc:kc + 1], -1.0)
                        nc.scalar.activation(at[:, kc * KCHUNK:(kc + 1) * KCHUNK], sps[:],
                                             AF.Exp, bias=negs[:, kc:kc + 1], scale=1.0,
                                             accum_out=dvec[:, kc:kc + 1])
                    # combine chunks: factors = exp(mrun_j - m_final)
                    fac = smp.tile([128, NKC], F32, tag="fac")
                    nc.scalar.activation(fac[:], mruns[:], AF.Exp,
                                         bias=negs[:, NKC - 1:NKC], scale=1.0)
                    dsc = smp.tile([128, NKC], F32, tag="dsc")
                    nc.vector.tensor_tensor(dsc[:], dvec[:], fac[:], OP.mult)
                    dtot = smp.tile([128, 1], F32, tag="dtot")
                    nc.vector.reduce_sum(dtot[:], dsc[:], axis=AX.X)
                    rd = smp.tile([128, 1], F32, tag="rd")
                    nc.vector.reciprocal(rd[:], dtot[:])
                    for kc in range(NKC - 1):
                        nc.vector.tensor_scalar_mul(at[:, kc * KCHUNK:(kc + 1) * KCHUNK],
                                                    at[:, kc * KCHUNK:(kc + 1) * KCHUNK],
                                                    fac[:, kc:kc + 1])
                    if pend is not None:
                        flush(pend)
                    pend = (at, rd, (t * PIX // 128 + j) * 128)
            flush(pend)

    nc.compile()
    return nc


_NC = None
_last_in_maps = None


def _get_nc():
    global _NC
    if _NC is None:
        _NC = build_nc()
    return _NC


def kernel(content_feat, style_feat, Wq, bq, Wk, bk, Wv, bv):
    content = np.asarray(content_feat, dtype=np.float32).reshape(B, C, HW)
    style = np.asarray(style_feat, dtype=np.float32).reshape(B, C, HW)
    wq_t = np.ascontiguousarray(np.asarray(Wq, dtype=np.float32).T)
    wk_t = np.ascontiguousarray(np.asarray(Wk, dtype=np.float32).T)
    wv_t = np.ascontiguousarray(np.asarray(Wv, dtype=np.float32).T)
    bq_p = np.ascontiguousarray(np.asarray(bq, dtype=np.float32).reshape(CS, 128).T)
    bk_p = np.ascontiguousarray(np.asarray(bk, dtype=np.float32).reshape(CS, 128).T)
    bv_v = np.ascontiguousarray(np.asarray(bv, dtype=np.float32))

    in_maps = []
    for core in range(8):
        b = core // 2
        half = core % 2
        # stats need the full 4096 content columns; the Q projection reads
        # chunks 0..3, so roll this core's half to the front
        xc_full = content[b]
        if half == 1:
            xc_full = np.concatenate([xc_full[:, QN:], xc_full[:, :QN]], axis=1)
        in_maps.append({
            "xc": np.ascontiguousarray(xc_full),
            "xs": np.ascontiguousarray(style[b]),
            "wq_t": wq_t, "wk_t": wk_t, "wv_t": wv_t,
            "bq_p": bq_p, "bk_p": bk_p, "bv_v": bv_v,
        })

    global _last_in_maps
    _last_in_maps = in_maps
    nc = _get_nc()
    res = run_bass_kernel_spmd(nc, in_maps, core_ids=list(range(8)))

    outf = np.empty((B, C, HW), dtype=np.float32)
    for core in range(8):
        b = core // 2
        half = core % 2
        ot = np.asarray(res.results[core]["out_t"])  # [QN, C]
        outf[b, :, half * QN:(half + 1) * QN] = ot.T
    return outf.reshape(B, C, H, W)


if __name__ == "__main__":
    rng = np.random.default_rng(0)
    inputs = {
        "content_feat": rng.standard_normal((B, C, H, W), dtype=np.float32),
        "style_feat": rng.standard_normal((B, C, H, W), dtype=np.float32),
        "Wq": rng.standard_normal((C, C), dtype=np.float32) * 0.05,
        "bq": rng.random(C, dtype=np.float32),
        "Wk": rng.standard_normal((C, C), dtype=np.float32) * 0.05,
        "bk": rng.random(C, dtype=np.float32),
        "Wv": rng.standard_normal((C, C), dtype=np.float32) * 0.05,
        "bv": rng.random(C, dtype=np.float32),
    }
    out = kernel(**inputs)
    print("kernel output:", out.shape, out.dtype, float(np.abs(out).max()))
